# revision 42
# baseline (speedup 1.0000x reference)
"""Trainium2 Bass kernel: 4-layer dense transformer (B=2, T=2048, D=512, H=8, V=32000).

Sharding (DP2 x TP4 over 8 cores): core c handles batch b=c//4, TP rank r=c%4.
Per core: 2 attention heads (2r, 2r+1) over the whole batch, a 512-token
sequence shard (tokens [512r, 512r+512)) of the residual stream for
Wo/LN/FFN, and an 8000-row vocab shard of the final projection.

Per layer: QKV+attention run on all 2048 tokens for the core's 2 heads
(keys need full T); AllGather #1 exchanges per-head attention outputs so
each core gets u[all 512 dims, :] and keeps only its 512-token slice for
Wo + residual + LN2 + FFN. The next layer's LN1 (or the final LN) is then
applied locally to the updated 512-token slice and AllGather #2
redistributes the normalized activations, which is all any core needs of
other cores' tokens (the raw residual h never has to be replicated).

Attention is computed in [head_dim, query] orientation: ps = V^T @ P with
a ones-column in V^T accumulating the softmax denominator, then a
row-broadcast reciprocal multiply writes uT directly -- no PE transposes.

Activations are kept transposed [D-partition, token-free]. Host-side prep
in kernel(): embedding gather + positional add, weight transposes, bf16
casts, per-core slicing. Device matmuls are bf16 with fp32 accumulation.
Logits are written bf16 and cast to f32 on the host.
"""
import sys
sys.path.insert(0, "/opt/trn_rl_repo")
import numpy as np
import ml_dtypes

import concourse.bass as bass
import concourse.mybir as mybir
import concourse.tile as tile
from concourse import bacc
from concourse.bass_utils import run_bass_kernel_spmd

F32 = mybir.dt.float32
BF16 = mybir.dt.bfloat16

N_CORES = 8
GROUPS = [[0, 1, 2, 3], [4, 5, 6, 7]]
P = 128
D = 512            # d_model
T = 2048           # tokens per batch (= per core for attention)
TS = 512           # tokens per core for Wo/LN/FFN (sequence shard)
H_PER = 2          # heads per core
DK = 64
L = 4              # layers
FF = 2048          # d_ff
VSH = 8000         # vocab shard per core
DC = D // P        # 4 D-chunks
TC = T // P        # 16 token chunks
TW = T // TS       # 4 token windows of 512
FC = FF // P       # 16 ff chunks
EPS = 1e-5


def build_nc_full():
    nc = bacc.Bacc("TRN2", target_bir_lowering=False, debug=False,
                   num_devices=N_CORES)
    h0T = nc.declare_dram_parameter("h0T", [D, TS], F32, isOutput=False)
    wqkvT = nc.declare_dram_parameter("wqkvT", [L, D, 3 * P], BF16, isOutput=False)
    woT = nc.declare_dram_parameter("woT", [L, P, D], BF16, isOutput=False)
    w1T = nc.declare_dram_parameter("w1T", [L, D, FF], BF16, isOutput=False)
    w2T = nc.declare_dram_parameter("w2T", [L, FF, D], BF16, isOutput=False)
    ln1g = nc.declare_dram_parameter("ln1g", [L, P, DC], F32, isOutput=False)
    ln1b = nc.declare_dram_parameter("ln1b", [L, P, DC], F32, isOutput=False)
    ln2g = nc.declare_dram_parameter("ln2g", [L, P, DC], F32, isOutput=False)
    ln2b = nc.declare_dram_parameter("ln2b", [L, P, DC], F32, isOutput=False)
    b1v = nc.declare_dram_parameter("b1v", [L, P, FC], F32, isOutput=False)
    b2v = nc.declare_dram_parameter("b2v", [L, P, DC], F32, isOutput=False)
    lnfg = nc.declare_dram_parameter("lnfg", [P, DC], F32, isOutput=False)
    lnfb = nc.declare_dram_parameter("lnfb", [P, DC], F32, isOutput=False)
    outwT = nc.declare_dram_parameter("outwT", [D, VSH], BF16, isOutput=False)
    logits = nc.declare_dram_parameter("logits", [T, VSH], BF16, isOutput=True)

    from contextlib import ExitStack
    with tile.TileContext(nc) as tc:
        with ExitStack() as ctx:
            ep = ctx.enter_context
            const = ep(tc.tile_pool(name="const", bufs=1))
            hpool = ep(tc.tile_pool(name="hpool", bufs=2))
            hmidp = ep(tc.tile_pool(name="hmid", bufs=1))
            awp = ep(tc.tile_pool(name="awp", bufs=1))
            awmyp = ep(tc.tile_pool(name="awmy", bufs=2))
            qkp = ep(tc.tile_pool(name="qkp", bufs=1))
            vxp = ep(tc.tile_pool(name="vx", bufs=1))
            utp = ep(tc.tile_pool(name="ut", bufs=1))
            utwp = ep(tc.tile_pool(name="utw", bufs=2))
            wgt = ep(tc.tile_pool(name="wgt", bufs=1))
            vecs = ep(tc.tile_pool(name="vecs", bufs=1))
            lnbig = ep(tc.tile_pool(name="lnbig", bufs=4))
            lnwin = ep(tc.tile_pool(name="lnwin", bufs=2))
            strow = ep(tc.tile_pool(name="strow", bufs=2))
            smallp = ep(tc.tile_pool(name="small", bufs=2))
            recp = ep(tc.tile_pool(name="recp", bufs=2))
            ptp = ep(tc.tile_pool(name="pt", bufs=17))
            z1p = ep(tc.tile_pool(name="z1w", bufs=1))
            a2p = ep(tc.tile_pool(name="a2p", bufs=1))
            dlt = ep(tc.tile_pool(name="dlt", bufs=1))
            owp = ep(tc.tile_pool(name="ow", bufs=8))
            psm = ep(tc.tile_pool(name="ps", bufs=3, space="PSUM"))
            psatt = ep(tc.tile_pool(name="psatt", bufs=2, space="PSUM"))
            psb = ep(tc.tile_pool(name="psb", bufs=2, space="PSUM"))
            pstat = ep(tc.tile_pool(name="pst", bufs=1, space="PSUM"))
            dram = ep(tc.tile_pool(name="dram", bufs=2, space="DRAM"))

            # ---- constants ----
            mean_lhs = const.tile([P, 1], F32, tag="mean_lhs")
            nc.gpsimd.memset(mean_lhs[:], 1.0 / D)
            ones_row = const.tile([1, P], F32, tag="ones_row")
            nc.gpsimd.memset(ones_row[:], 1.0)
            eps_t = const.tile([P, 1], F32, tag="eps_t")
            nc.gpsimd.memset(eps_t[:], EPS)

            # vx tiles: [keys, DK val dims + ones column]; ones set once.
            vx = [[vxp.tile([P, DK + 1], BF16, tag=f"vx{kj}_{h}",
                            name=f"vx{kj}_{h}")
                   for h in range(H_PER)] for kj in range(TC)]
            for kj in range(TC):
                for h in range(H_PER):
                    nc.gpsimd.memset(vx[kj][h][:, DK:DK + 1], 1.0)

            # ---- all LN / bias vectors, loaded once ----
            def vload(src, l, w, tag):
                t = vecs.tile([P, w], F32, tag=tag)
                nc.gpsimd.dma_start(t[:], src[l] if l is not None else src[:, :])
                return t

            g1v = [vload(ln1g, l, DC, f"g1_{l}") for l in range(L)]
            b1lv = [vload(ln1b, l, DC, f"bb1_{l}") for l in range(L)]
            g2v = [vload(ln2g, l, DC, f"g2_{l}") for l in range(L)]
            b2lv = [vload(ln2b, l, DC, f"bb2_{l}") for l in range(L)]
            fb1v = [vload(b1v, l, FC, f"fb1_{l}") for l in range(L)]
            fb2v = [vload(b2v, l, DC, f"fb2_{l}") for l in range(L)]
            gfv = vload(lnfg, None, DC, "gf")
            bfv = vload(lnfb, None, DC, "bf")

            # ---- initial residual slice ----
            h_cur = [hpool.tile([P, TS], F32, tag=f"h{c}", name=f"h{c}_init")
                     for c in range(DC)]
            for c in range(DC):
                nc.sync.dma_start(h_cur[c][:], h0T[c * P:(c + 1) * P, :])

            def ln_apply(ins, g_t, b_t, outs, name):
                """LayerNorm over D: ins/outs are lists of DC [P,TS] tiles."""
                s01 = lnbig.tile([P, TS], F32, tag="lnbig", name=f"{name}_s01")
                s23 = lnbig.tile([P, TS], F32, tag="lnbig", name=f"{name}_s23")
                nc.vector.tensor_add(s01[:], ins[0][:], ins[1][:])
                nc.vector.tensor_add(s23[:], ins[2][:], ins[3][:])
                nc.vector.tensor_add(s01[:], s01[:], s23[:])
                q0 = lnbig.tile([P, TS], F32, tag="lnbig", name=f"{name}_q0")
                q1 = lnbig.tile([P, TS], F32, tag="lnbig", name=f"{name}_q1")
                # squares chain on Pool so it runs beside the DVE sum chain
                nc.gpsimd.tensor_tensor(out=q0[:], in0=ins[0][:], in1=ins[0][:],
                                        op=mybir.AluOpType.mult)
                for c in range(1, DC):
                    nc.gpsimd.tensor_tensor(out=q1[:], in0=ins[c][:],
                                            in1=ins[c][:],
                                            op=mybir.AluOpType.mult)
                    nc.gpsimd.tensor_add(q0[:], q0[:], q1[:])
                mp = pstat.tile([1, TS], F32, space="PSUM", tag="st")
                nc.tensor.matmul(mp[:], mean_lhs[:], s01[:], start=True, stop=True)
                mu_row = strow.tile([1, TS], F32, tag="mu_row")
                nc.scalar.copy(mu_row[:], mp[:])
                mp2 = pstat.tile([1, TS], F32, space="PSUM", tag="st")
                nc.tensor.matmul(mp2[:], mean_lhs[:], q0[:], start=True, stop=True)
                ms_row = strow.tile([1, TS], F32, tag="ms_row")
                nc.scalar.copy(ms_row[:], mp2[:])
                bp = psm.tile([P, TS], F32, space="PSUM", tag="mm")
                nc.tensor.matmul(bp[:], ones_row[:], mu_row[:], start=True, stop=True)
                mu_bc = lnwin.tile([P, TS], F32, tag="mu_bc")
                nc.vector.tensor_copy(mu_bc[:], bp[:])
                bp2 = psm.tile([P, TS], F32, space="PSUM", tag="mm")
                nc.tensor.matmul(bp2[:], ones_row[:], ms_row[:], start=True, stop=True)
                rstd = lnwin.tile([P, TS], F32, tag="rstd")
                nc.vector.tensor_tensor(out=rstd[:], in0=mu_bc[:], in1=mu_bc[:],
                                        op=mybir.AluOpType.mult)
                nc.vector.tensor_tensor(out=rstd[:], in0=bp2[:], in1=rstd[:],
                                        op=mybir.AluOpType.subtract)
                # rstd = exp(-0.5*ln(var+eps)); Ln/Exp share an ACT table
                # (unlike Sqrt) so this avoids 1.3us table reloads around the
                # attention Exp phase, and drops the DVE reciprocal.
                nc.scalar.activation(rstd[:], rstd[:],
                                     mybir.ActivationFunctionType.Ln,
                                     bias=eps_t[:])
                nc.scalar.activation(rstd[:], rstd[:],
                                     mybir.ActivationFunctionType.Exp,
                                     bias=0.0, scale=-0.5)
                for c in range(DC):
                    eng = nc.vector if c % 2 == 0 else nc.gpsimd
                    tt = smallp.tile([P, TS], F32, tag="ln_app")
                    eng.tensor_tensor(out=tt[:], in0=ins[c][:],
                                      in1=mu_bc[:],
                                      op=mybir.AluOpType.subtract)
                    eng.tensor_tensor(out=tt[:], in0=tt[:], in1=rstd[:],
                                      op=mybir.AluOpType.mult)
                    nc.vector.tensor_scalar(
                        out=outs[c][:], in0=tt[:],
                        scalar1=g_t[:, c:c + 1], scalar2=b_t[:, c:c + 1],
                        op0=mybir.AluOpType.mult, op1=mybir.AluOpType.add)

            def emit_aw_ag(h_tiles, g_t, b_t, name):
                """LN local slice -> AllGather -> per-(chunk,window) aw tiles."""
                aw_my = [awmyp.tile([P, TS], BF16, tag=f"awmy{c}",
                                    name=f"awmy{c}_{name}") for c in range(DC)]
                ln_apply(h_tiles, g_t, b_t, aw_my, name)
                ag_in = dram.tile([D, TS], BF16, tag="ag2_in", name=f"agi_{name}")
                ag_out = dram.tile([4 * D, TS], BF16, tag="ag2_out",
                                   name=f"ago_{name}")
                for c in range(DC):
                    nc.sync.dma_start(ag_in[c * P:(c + 1) * P, :], aw_my[c][:])
                nc.gpsimd.collective_compute(
                    "AllGather", mybir.AluOpType.bypass,
                    replica_groups=GROUPS,
                    ins=[ag_in[:].opt()], outs=[ag_out[:].opt()])
                aw = [[awp.tile([P, TS], BF16, tag=f"aw{c}_{r}",
                                name=f"aw{c}_{r}_{name}")
                       for r in range(TW)] for c in range(DC)]
                for r in range(TW):
                    for c in range(DC):
                        nc.sync.dma_start(
                            aw[c][r][:],
                            ag_out[r * D + c * P:r * D + (c + 1) * P, :])
                return aw

            # layer-0 LN1 + gather
            aw = emit_aw_ag(h_cur, g1v[0], b1lv[0], "l0in")

            for l in range(L):
                # ---- weights for this layer (loads overlap prior compute) ----
                wq_sb = [wgt.tile([P, 3 * P], BF16, tag=f"wq{k}", name=f"wq{k}_{l}")
                         for k in range(DC)]
                wo_sb = wgt.tile([P, D], BF16, tag="wo", name=f"wo_{l}")
                w1_sb = [wgt.tile([P, FF], BF16, tag=f"w1{k}", name=f"w1{k}_{l}")
                         for k in range(DC)]
                w2_sb = [wgt.tile([P, D], BF16, tag=f"w2{k}", name=f"w2{k}_{l}")
                         for k in range(FC)]
                nc.gpsimd.dma_start(wo_sb[:], woT[l])
                for k in range(DC):
                    nc.gpsimd.dma_start(wq_sb[k][:], wqkvT[l, k * P:(k + 1) * P, :])
                    nc.gpsimd.dma_start(w1_sb[k][:], w1T[l, k * P:(k + 1) * P, :])
                for k in range(FC):
                    nc.gpsimd.dma_start(w2_sb[k][:], w2T[l, k * P:(k + 1) * P, :])

                # ---- Q, K projections (all T), windowed ----
                qk_sb = [qkp.tile([P, T], BF16, tag=f"qk{m}", name=f"qk{m}_{l}")
                         for m in range(2)]
                for w in range(TW):
                    for m in range(2):
                        pp = psm.tile([P, TS], F32, space="PSUM", tag="mm")
                        for k in range(DC):
                            nc.tensor.matmul(
                                pp[:], wq_sb[k][:, m * P:(m + 1) * P], aw[k][w][:],
                                start=(k == 0), stop=(k == DC - 1))
                        if m == 0:
                            nc.scalar.copy(qk_sb[m][:, w * TS:(w + 1) * TS], pp[:])
                        else:
                            nc.vector.tensor_copy(
                                qk_sb[m][:, w * TS:(w + 1) * TS], pp[:])

                # ---- V^T tiles (token-partition), both heads at once ----
                for kj in range(TC):
                    wj, jj = kj // 4, kj % 4
                    vt = psm.tile([P, TS], F32, space="PSUM", tag="mm")
                    for k in range(DC):
                        nc.tensor.matmul(
                            vt[:, :P],
                            aw[k][wj][:, jj * P:(jj + 1) * P],
                            wq_sb[k][:, 2 * P:3 * P],
                            start=(k == 0), stop=(k == DC - 1))
                    nc.scalar.copy(vx[kj][0][:, :DK], vt[:, :DK])
                    nc.vector.tensor_copy(vx[kj][1][:, :DK], vt[:, DK:2 * DK])

                # ---- attention: scores -> exp -> mask -> (V^T|1) @ P ----
                # After each window, the partial Wo product (contraction over
                # this core's 128 head dims) is computed and staged for a
                # ReduceScatter that sums partials over ranks and scatters
                # token windows -- core r receives Wo-out[:, r's tokens].
                rs_in = dram.tile([4 * D, TS], BF16, tag="rs_in",
                                  name=f"rsi_{l}")
                rs_out = dram.tile([D, TS], BF16, tag="rs_out",
                                   name=f"rso_{l}")
                uT = utp.tile([P, T], BF16, tag="uT", name=f"uT_{l}")
                for w in range(TW):
                    qsl = slice(w * TS, (w + 1) * TS)
                    for h in range(H_PER):
                        hs = slice(h * DK, (h + 1) * DK)
                        nkj = 4 * (w + 1)
                        pts = []
                        for kj in range(nkj):
                            sp = psm.tile([P, TS], F32, space="PSUM", tag="mm")
                            nc.tensor.matmul(
                                sp[:], qk_sb[1][hs, kj * P:(kj + 1) * P],
                                qk_sb[0][hs, qsl], start=True, stop=True)
                            pt = ptp.tile([P, TS], BF16, tag="pt")
                            nc.scalar.activation(
                                pt[:], sp[:], mybir.ActivationFunctionType.Exp,
                                bias=0.0, scale=0.125)
                            if kj >= 4 * w:
                                off = kj * P - w * TS
                                nc.gpsimd.affine_select(
                                    out=pt[:], in_=pt[:],
                                    compare_op=mybir.AluOpType.is_ge,
                                    fill=0.0, base=-off,
                                    pattern=[[1, TS]], channel_multiplier=-1)
                            pts.append(pt)
                        pa = psatt.tile([DK + 1, TS], F32, space="PSUM", tag="att")
                        for kj in range(nkj):
                            nc.tensor.matmul(
                                pa[:], vx[kj][h][:], pts[kj][:],
                                start=(kj == 0), stop=(kj == nkj - 1))
                        den = strow.tile([1, TS], F32, tag="den")
                        nc.vector.tensor_copy(den[:], pa[DK:DK + 1, :])
                        pb = psb.tile([DK, TS], F32, space="PSUM", tag="bc")
                        nc.tensor.matmul(pb[:], ones_row[:, :DK], den[:],
                                         start=True, stop=True)
                        rec = recp.tile([DK, TS], F32, tag="rec")
                        nc.vector.reciprocal(rec[:], pb[:])
                        nc.vector.tensor_tensor(
                            out=uT[hs, qsl], in0=pa[:DK, :], in1=rec[:],
                            op=mybir.AluOpType.mult)
                    # partial Wo for this window, staged to the RS buffer
                    for m in range(DC):
                        pw = psm.tile([P, TS], F32, space="PSUM", tag="mm")
                        nc.tensor.matmul(
                            pw[:], wo_sb[:, m * P:(m + 1) * P], uT[:, qsl],
                            start=True, stop=True)
                        wop = utwp.tile([P, TS], BF16, tag="wop",
                                        name=f"wop{l}_{w}_{m}")
                        nc.vector.tensor_copy(wop[:], pw[:])
                        nc.sync.dma_start(
                            rs_in[w * D + m * P:w * D + (m + 1) * P, :], wop[:])

                nc.gpsimd.collective_compute(
                    "ReduceScatter", mybir.AluOpType.add,
                    replica_groups=GROUPS,
                    ins=[rs_in[:].opt()], outs=[rs_out[:].opt()])

                # ---- residual on my token slice ----
                delta = [dlt.tile([P, TS], BF16, tag=f"dl{m}", name=f"dl{m}_{l}")
                         for m in range(DC)]
                hmid = [hmidp.tile([P, TS], F32, tag=f"hm{m}", name=f"hm{m}_{l}")
                        for m in range(DC)]
                for m in range(DC):
                    nc.sync.dma_start(delta[m][:], rs_out[m * P:(m + 1) * P, :])
                    nc.vector.tensor_add(hmid[m][:], h_cur[m][:], delta[m][:])

                # ---- LN2 + FFN on my token slice ----
                a2 = [a2p.tile([P, TS], BF16, tag=f"a2_{c}", name=f"a2{c}_{l}")
                      for c in range(DC)]
                ln_apply(hmid, g2v[l], b2lv[l], a2, f"ln2_{l}")
                z1g = [z1p.tile([P, TS], BF16, tag=f"z1_{m}", name=f"z1{m}_{l}")
                       for m in range(FC)]
                for m in range(FC):
                    pp = psm.tile([P, TS], F32, space="PSUM", tag="mm")
                    for k in range(DC):
                        nc.tensor.matmul(
                            pp[:], w1_sb[k][:, m * P:(m + 1) * P], a2[k][:],
                            start=(k == 0), stop=(k == DC - 1))
                    nc.scalar.activation(
                        z1g[m][:], pp[:], mybir.ActivationFunctionType.Gelu,
                        bias=fb1v[l][:, m:m + 1])
                h_new = [hpool.tile([P, TS], F32, tag=f"h{c}", name=f"h{c}_{l + 1}")
                         for c in range(DC)]
                for md in range(DC):
                    pp = psm.tile([P, TS], F32, space="PSUM", tag="mm")
                    for k in range(FC):
                        nc.tensor.matmul(
                            pp[:], w2_sb[k][:, md * P:(md + 1) * P], z1g[k][:],
                            start=(k == 0), stop=(k == FC - 1))
                    tt = smallp.tile([P, TS], F32, tag="ffn_out")
                    nc.vector.tensor_scalar(
                        out=tt[:], in0=pp[:], scalar1=fb2v[l][:, md:md + 1],
                        scalar2=None, op0=mybir.AluOpType.add)
                    nc.vector.tensor_add(h_new[md][:], hmid[md][:], tt[:])

                # ---- next-layer LN1 (or final LN) + AllGather ----
                if l + 1 < L:
                    aw = emit_aw_ag(h_new, g1v[l + 1], b1lv[l + 1], f"l{l + 1}in")
                else:
                    aw = emit_aw_ag(h_new, gfv, bfv, "fin")
                h_cur = h_new

            # ---- vocab-shard projection from gathered final-LN activations ----
            NV = 500
            for vc in range(VSH // NV):
                ow_sb = [owp.tile([P, NV], BF16, tag="ow", name=f"ow{vc}_{k2}")
                         for k2 in range(DC)]
                for k in range(DC):
                    nc.gpsimd.dma_start(
                        ow_sb[k][:],
                        outwT[k * P:(k + 1) * P, vc * NV:(vc + 1) * NV])
                for tcx in range(TC):
                    r, j = tcx // 4, tcx % 4
                    pp = psm.tile([P, TS], F32, space="PSUM", tag="mm")
                    for k in range(DC):
                        nc.tensor.matmul(
                            pp[:, :NV], aw[k][r][:, j * P:(j + 1) * P],
                            ow_sb[k][:], start=(k == 0), stop=(k == DC - 1))
                    lo = smallp.tile([P, NV], BF16, tag="lo", name=f"lo{vc}_{tcx}")
                    if tcx % 2 == 0:
                        nc.scalar.copy(lo[:], pp[:, :NV])
                    else:
                        nc.vector.tensor_copy(lo[:], pp[:, :NV])
                    nc.sync.dma_start(
                        logits[tcx * P:(tcx + 1) * P, vc * NV:(vc + 1) * NV],
                        lo[:])
    nc.compile()
    return nc


_NC_CACHE = None


def _get_nc():
    global _NC_CACHE
    if _NC_CACHE is None:
        _NC_CACHE = build_nc_full()
    return _NC_CACHE


def _vec_tile(v, chunks):
    # [chunks*128] -> [128, chunks] with [p, c] = v[c*128+p]
    return np.ascontiguousarray(np.asarray(v, np.float32).reshape(chunks, P).T)


def prepare_in_maps(inputs):
    return _prep(**inputs)


def _prep(x, embed_w, pos_w, ln1_g, ln1_b, Wqkv, Wo, ln2_g, ln2_b,
          W1, b1, W2, b2, lnf_g, lnf_b, out_w):
    x = np.asarray(x)
    embed_w = np.asarray(embed_w, np.float32)
    pos_w = np.asarray(pos_w, np.float32)
    Wqkv = np.asarray(Wqkv, np.float32)
    bf = ml_dtypes.bfloat16
    woT_full = np.ascontiguousarray(np.asarray(Wo).transpose(0, 2, 1)).astype(bf)
    w1T = np.ascontiguousarray(np.asarray(W1).transpose(0, 2, 1)).astype(bf)
    w2T = np.ascontiguousarray(np.asarray(W2).transpose(0, 2, 1)).astype(bf)
    ln_tiles = {
        "ln1g": np.stack([_vec_tile(np.asarray(ln1_g)[l], DC) for l in range(L)]),
        "ln1b": np.stack([_vec_tile(np.asarray(ln1_b)[l], DC) for l in range(L)]),
        "ln2g": np.stack([_vec_tile(np.asarray(ln2_g)[l], DC) for l in range(L)]),
        "ln2b": np.stack([_vec_tile(np.asarray(ln2_b)[l], DC) for l in range(L)]),
        "b1v": np.stack([_vec_tile(np.asarray(b1)[l], FC) for l in range(L)]),
        "b2v": np.stack([_vec_tile(np.asarray(b2)[l], DC) for l in range(L)]),
        "lnfg": _vec_tile(lnf_g, DC),
        "lnfb": _vec_tile(lnf_b, DC),
    }
    in_maps = []
    for c in range(N_CORES):
        b, r = c // 4, c % 4
        h0 = embed_w[x[b]] + pos_w[:T]                       # [T, D]
        h0T = np.ascontiguousarray(
            h0[r * TS:(r + 1) * TS].T).astype(np.float32)    # [D, TS]
        heads = [2 * r, 2 * r + 1]
        rows = np.concatenate([
            np.r_[heads[0] * DK:(heads[0] + 1) * DK,
                  heads[1] * DK:(heads[1] + 1) * DK] + w * D
            for w in range(3)])
        wqkvT = np.ascontiguousarray(
            Wqkv[:, rows, :].transpose(0, 2, 1)).astype(bf)  # [L, 512, 384]
        woT = np.ascontiguousarray(
            woT_full[:, r * P:(r + 1) * P, :])               # [L, 128, 512]
        outwT = np.ascontiguousarray(
            np.asarray(out_w)[r * VSH:(r + 1) * VSH].T).astype(bf)
        m = {"h0T": h0T, "wqkvT": wqkvT, "woT": woT, "w1T": w1T, "w2T": w2T,
             "outwT": outwT}
        m.update(ln_tiles)
        in_maps.append(m)
    return in_maps


def kernel(**inputs):
    nc = _get_nc()
    in_maps = prepare_in_maps(inputs)
    res = run_bass_kernel_spmd(nc, in_maps, list(range(N_CORES)))
    out = np.empty((2, T, 4 * VSH), np.float32)
    for c in range(N_CORES):
        b, r = c // 4, c % 4
        out[b, :, r * VSH:(r + 1) * VSH] = \
            np.asarray(res.results[c]["logits"]).astype(np.float32)
    return out


# revision 60
# speedup vs baseline: 1.3797x; 1.3797x over previous
"""Trainium2 Bass kernel: 4-layer dense transformer (B=2, T=2048, D=512, H=8, V=32000).

Sharding (DP2 x TP4 over 8 cores): core c handles batch b=c//4, TP rank r=c%4.
Per core: 2 attention heads (2r, 2r+1) over the whole batch, a 512-token
sequence shard (tokens [512r, 512r+512)) of the residual stream for
Wo/LN/FFN, and an 8000-row vocab shard of the final projection.

Per layer: QKV+attention run on all 2048 tokens for the core's 2 heads
(keys need full T); AllGather #1 exchanges per-head attention outputs so
each core gets u[all 512 dims, :] and keeps only its 512-token slice for
Wo + residual + LN2 + FFN. The next layer's LN1 (or the final LN) is then
applied locally to the updated 512-token slice and AllGather #2
redistributes the normalized activations, which is all any core needs of
other cores' tokens (the raw residual h never has to be replicated).

Attention is computed in [head_dim, query] orientation: ps = V^T @ P with
a ones-column in V^T accumulating the softmax denominator, then a
row-broadcast reciprocal multiply writes uT directly -- no PE transposes.

Activations are kept transposed [D-partition, token-free]. Host-side prep
in kernel(): embedding gather + positional add, weight transposes, bf16
casts, per-core slicing. Device matmuls are bf16 with fp32 accumulation.
Logits are written bf16 and cast to f32 on the host.
"""
import sys
sys.path.insert(0, "/opt/trn_rl_repo")
import numpy as np
import ml_dtypes

import concourse.bass as bass
import concourse.mybir as mybir
import concourse.tile as tile
from concourse import bacc
from concourse.bass_utils import run_bass_kernel_spmd

F32 = mybir.dt.float32
BF16 = mybir.dt.bfloat16

N_CORES = 8
GROUPS = [[0, 1, 2, 3], [4, 5, 6, 7]]
P = 128
D = 512            # d_model
T = 2048           # tokens per batch (= per core for attention)
TS = 512           # tokens per core for Wo/LN/FFN (sequence shard)
H_PER = 2          # heads per core
DK = 64
L = 4              # layers
FF = 2048          # d_ff
VSH = 8000         # vocab shard per core
DC = D // P        # 4 D-chunks
TC = T // P        # 16 token chunks
TW = T // TS       # 4 token windows of 512
FC = FF // P       # 16 ff chunks
EPS = 1e-5


def build_nc_full():
    nc = bacc.Bacc("TRN2", target_bir_lowering=False, debug=False,
                   num_devices=N_CORES)
    h0T = nc.declare_dram_parameter("h0T", [D, TS], F32, isOutput=False)
    wqkvT = nc.declare_dram_parameter("wqkvT", [L, D, 3 * P], BF16, isOutput=False)
    woT = nc.declare_dram_parameter("woT", [L, P, D], BF16, isOutput=False)
    w1T = nc.declare_dram_parameter("w1T", [L, D, FF], BF16, isOutput=False)
    w2T = nc.declare_dram_parameter("w2T", [L, FF, D], BF16, isOutput=False)
    ln1g = nc.declare_dram_parameter("ln1g", [L, P, DC], F32, isOutput=False)
    ln1b = nc.declare_dram_parameter("ln1b", [L, P, DC], F32, isOutput=False)
    ln2g = nc.declare_dram_parameter("ln2g", [L, P, DC], F32, isOutput=False)
    ln2b = nc.declare_dram_parameter("ln2b", [L, P, DC], F32, isOutput=False)
    b1v = nc.declare_dram_parameter("b1v", [L, P, FC], F32, isOutput=False)
    b2v = nc.declare_dram_parameter("b2v", [L, P, DC], F32, isOutput=False)
    lnfg = nc.declare_dram_parameter("lnfg", [P, DC], F32, isOutput=False)
    lnfb = nc.declare_dram_parameter("lnfb", [P, DC], F32, isOutput=False)
    outwT = nc.declare_dram_parameter("outwT", [D, VSH], BF16, isOutput=False)
    logits = nc.declare_dram_parameter("logits", [T, VSH], BF16, isOutput=True)

    from contextlib import ExitStack
    with tile.TileContext(nc) as tc:
        with ExitStack() as ctx:
            ep = ctx.enter_context
            const = ep(tc.tile_pool(name="const", bufs=1))
            hpool = ep(tc.tile_pool(name="hpool", bufs=2))
            hmidp = ep(tc.tile_pool(name="hmid", bufs=1))
            awp = ep(tc.tile_pool(name="awp", bufs=1))
            awmyp = ep(tc.tile_pool(name="awmy", bufs=2))
            qkp = ep(tc.tile_pool(name="qkp", bufs=1))
            vxp = ep(tc.tile_pool(name="vx", bufs=1))
            utp = ep(tc.tile_pool(name="ut", bufs=1))
            utwp = ep(tc.tile_pool(name="utw", bufs=2))
            wgt = ep(tc.tile_pool(name="wgt", bufs=1))
            vecs = ep(tc.tile_pool(name="vecs", bufs=1))
            lnbig = ep(tc.tile_pool(name="lnbig", bufs=4))
            lnwin = ep(tc.tile_pool(name="lnwin", bufs=2))
            strow = ep(tc.tile_pool(name="strow", bufs=2))
            smallp = ep(tc.tile_pool(name="small", bufs=2))
            recp = ep(tc.tile_pool(name="recp", bufs=2))
            ptp = ep(tc.tile_pool(name="pt", bufs=9))
            z1p = ep(tc.tile_pool(name="z1w", bufs=1))
            a2p = ep(tc.tile_pool(name="a2p", bufs=1))
            dlt = ep(tc.tile_pool(name="dlt", bufs=1))
            owp = ep(tc.tile_pool(name="ow", bufs=8))
            psm = ep(tc.tile_pool(name="ps", bufs=2, space="PSUM"))
            pss = ep(tc.tile_pool(name="pss", bufs=2, space="PSUM"))
            psatt = ep(tc.tile_pool(name="psatt", bufs=2, space="PSUM"))
            dram = ep(tc.tile_pool(name="dram", bufs=2, space="DRAM"))

            # ---- constants ----
            mean_lhs = const.tile([P, 1], F32, tag="mean_lhs")
            nc.gpsimd.memset(mean_lhs[:], 1.0 / D)
            ones_row = const.tile([1, P], F32, tag="ones_row")
            nc.gpsimd.memset(ones_row[:], 1.0)
            eps_t = const.tile([P, 1], F32, tag="eps_t")
            nc.gpsimd.memset(eps_t[:], EPS)

            # vx tiles: [keys, DK val dims + ones column]; ones set once.
            vx = [[vxp.tile([P, DK + 1], BF16, tag=f"vx{kj}_{h}",
                            name=f"vx{kj}_{h}")
                   for h in range(H_PER)] for kj in range(TC)]
            for kj in range(TC):
                for h in range(H_PER):
                    nc.gpsimd.memset(vx[kj][h][:, DK:DK + 1], 1.0)

            # ---- all LN / bias vectors, loaded once ----
            def vload(src, l, w, tag):
                t = vecs.tile([P, w], F32, tag=tag)
                nc.gpsimd.dma_start(t[:], src[l] if l is not None else src[:, :])
                return t

            g1v = [vload(ln1g, l, DC, f"g1_{l}") for l in range(L)]
            b1lv = [vload(ln1b, l, DC, f"bb1_{l}") for l in range(L)]
            g2v = [vload(ln2g, l, DC, f"g2_{l}") for l in range(L)]
            b2lv = [vload(ln2b, l, DC, f"bb2_{l}") for l in range(L)]
            fb1v = [vload(b1v, l, FC, f"fb1_{l}") for l in range(L)]
            fb2v = [vload(b2v, l, DC, f"fb2_{l}") for l in range(L)]
            gfv = vload(lnfg, None, DC, "gf")
            bfv = vload(lnfb, None, DC, "bf")

            # ---- initial residual slice ----
            h_cur = [hpool.tile([P, TS], F32, tag=f"h{c}", name=f"h{c}_init")
                     for c in range(DC)]
            for c in range(DC):
                nc.sync.dma_start(h_cur[c][:], h0T[c * P:(c + 1) * P, :])

            def ln_apply(ins, g_t, b_t, outs, name):
                """LayerNorm over D: ins/outs are lists of DC [P,TS] tiles.

                Processed as two independent 256-token half-chains (separate
                pool tags) so the stats->apply dependency pipelines at half
                granularity across DVE/GPSIMD/ACT/PE."""
                H2 = TS // 2
                for hf in range(2):
                    sl = slice(hf * H2, (hf + 1) * H2)
                    s01 = lnbig.tile([P, H2], F32, tag=f"lnbig{hf}",
                                     name=f"{name}_s01_{hf}")
                    s23 = lnbig.tile([P, H2], F32, tag=f"lnbig{hf}",
                                     name=f"{name}_s23_{hf}")
                    nc.vector.tensor_add(s01[:], ins[0][:, sl], ins[1][:, sl])
                    nc.vector.tensor_add(s23[:], ins[2][:, sl], ins[3][:, sl])
                    nc.vector.tensor_add(s01[:], s01[:], s23[:])
                    q0 = lnbig.tile([P, H2], F32, tag=f"lnbig{hf}",
                                    name=f"{name}_q0_{hf}")
                    q1 = lnbig.tile([P, H2], F32, tag=f"lnbig{hf}",
                                    name=f"{name}_q1_{hf}")
                    # squares chain on Pool, beside the DVE sum chain
                    nc.gpsimd.tensor_tensor(out=q0[:], in0=ins[0][:, sl],
                                            in1=ins[0][:, sl],
                                            op=mybir.AluOpType.mult)
                    for c in range(1, DC):
                        nc.gpsimd.tensor_tensor(out=q1[:], in0=ins[c][:, sl],
                                                in1=ins[c][:, sl],
                                                op=mybir.AluOpType.mult)
                        nc.gpsimd.tensor_add(q0[:], q0[:], q1[:])
                    mp = psm.tile([P, TS], F32, space="PSUM", tag="mm",
                                  name=f"{name}_mp_{hf}")
                    nc.tensor.matmul(mp[0:1, :H2], mean_lhs[:], s01[:],
                                     start=True, stop=True)
                    mu_row = strow.tile([1, H2], F32, tag=f"mu_row{hf}",
                                        name=f"{name}_mu_{hf}")
                    nc.scalar.copy(mu_row[:], mp[0:1, :H2])
                    mp2 = psm.tile([P, TS], F32, space="PSUM", tag="mm",
                                   name=f"{name}_mp2_{hf}")
                    nc.tensor.matmul(mp2[0:1, :H2], mean_lhs[:], q0[:],
                                     start=True, stop=True)
                    ms_row = strow.tile([1, H2], F32, tag=f"ms_row{hf}",
                                        name=f"{name}_ms_{hf}")
                    nc.scalar.copy(ms_row[:], mp2[0:1, :H2])
                    bp = psm.tile([P, TS], F32, space="PSUM", tag="mm",
                                  name=f"{name}_bp_{hf}")
                    nc.tensor.matmul(bp[:, :H2], ones_row[:], mu_row[:],
                                     start=True, stop=True)
                    mu_bc = lnwin.tile([P, H2], F32, tag=f"mu_bc{hf}",
                                       name=f"{name}_mub_{hf}")
                    nc.vector.tensor_copy(mu_bc[:], bp[:, :H2])
                    bp2 = psm.tile([P, TS], F32, space="PSUM", tag="mm",
                                   name=f"{name}_bp2_{hf}")
                    nc.tensor.matmul(bp2[:, :H2], ones_row[:], ms_row[:],
                                     start=True, stop=True)
                    rstd = lnwin.tile([P, H2], F32, tag=f"rstd{hf}",
                                      name=f"{name}_rs_{hf}")
                    nc.vector.tensor_tensor(out=rstd[:], in0=mu_bc[:],
                                            in1=mu_bc[:],
                                            op=mybir.AluOpType.mult)
                    nc.vector.tensor_tensor(out=rstd[:], in0=bp2[:, :H2],
                                            in1=rstd[:],
                                            op=mybir.AluOpType.subtract)
                    # rstd = exp(-0.5*ln(var+eps)); Ln/Exp share an ACT table
                    # (unlike Sqrt) so no 1.3us table reloads near attention
                    nc.scalar.activation(rstd[:], rstd[:],
                                         mybir.ActivationFunctionType.Ln,
                                         bias=eps_t[:])
                    nc.scalar.activation(rstd[:], rstd[:],
                                         mybir.ActivationFunctionType.Exp,
                                         bias=0.0, scale=-0.5)
                    for c in range(DC):
                        eng = nc.vector if c % 2 == 0 else nc.gpsimd
                        tt = smallp.tile([P, H2], F32, tag=f"ln_app{hf}",
                                         name=f"{name}_tt{c}_{hf}")
                        eng.tensor_tensor(out=tt[:], in0=ins[c][:, sl],
                                          in1=mu_bc[:],
                                          op=mybir.AluOpType.subtract)
                        eng.tensor_tensor(out=tt[:], in0=tt[:], in1=rstd[:],
                                          op=mybir.AluOpType.mult)
                        nc.vector.tensor_scalar(
                            out=outs[c][:, sl], in0=tt[:],
                            scalar1=g_t[:, c:c + 1], scalar2=b_t[:, c:c + 1],
                            op0=mybir.AluOpType.mult, op1=mybir.AluOpType.add)

            def emit_aw_ag(h_tiles, g_t, b_t, name):
                """LN local slice -> AllGather -> per-(chunk,window) aw tiles."""
                aw_my = [awmyp.tile([P, TS], BF16, tag=f"awmy{c}",
                                    name=f"awmy{c}_{name}") for c in range(DC)]
                ln_apply(h_tiles, g_t, b_t, aw_my, name)
                ag_in = dram.tile([D, TS], BF16, tag="ag2_in", name=f"agi_{name}")
                ag_out = dram.tile([4 * D, TS], BF16, tag="ag2_out",
                                   name=f"ago_{name}")
                for c in range(DC):
                    nc.sync.dma_start(ag_in[c * P:(c + 1) * P, :], aw_my[c][:])
                nc.gpsimd.collective_compute(
                    "AllGather", mybir.AluOpType.bypass,
                    replica_groups=GROUPS,
                    ins=[ag_in[:].opt()], outs=[ag_out[:].opt()])
                aw = [[awp.tile([P, TS], BF16, tag=f"aw{c}_{r}",
                                name=f"aw{c}_{r}_{name}")
                       for r in range(TW)] for c in range(DC)]
                for r in range(TW):
                    for c in range(DC):
                        nc.sync.dma_start(
                            aw[c][r][:],
                            ag_out[r * D + c * P:r * D + (c + 1) * P, :])
                return aw

            # layer-0 LN1 + gather
            aw = emit_aw_ag(h_cur, g1v[0], b1lv[0], "l0in")

            for l in range(L):
                # ---- weights for this layer (loads overlap prior compute) ----
                wq_sb = [wgt.tile([P, 3 * P], BF16, tag=f"wq{k}", name=f"wq{k}_{l}")
                         for k in range(DC)]
                wo_sb = wgt.tile([P, D], BF16, tag="wo", name=f"wo_{l}")
                w1_sb = [wgt.tile([P, FF], BF16, tag=f"w1{k}", name=f"w1{k}_{l}")
                         for k in range(DC)]
                w2_sb = [wgt.tile([P, D], BF16, tag=f"w2{k}", name=f"w2{k}_{l}")
                         for k in range(FC)]
                nc.gpsimd.dma_start(wo_sb[:], woT[l])
                for k in range(DC):
                    nc.gpsimd.dma_start(wq_sb[k][:], wqkvT[l, k * P:(k + 1) * P, :])
                    nc.gpsimd.dma_start(w1_sb[k][:], w1T[l, k * P:(k + 1) * P, :])
                for k in range(FC):
                    nc.gpsimd.dma_start(w2_sb[k][:], w2T[l, k * P:(k + 1) * P, :])

                # ---- Q, K projections (all T), windowed ----
                qk_sb = [qkp.tile([P, T], BF16, tag=f"qk{m}", name=f"qk{m}_{l}")
                         for m in range(2)]
                for w in range(TW):
                    for m in range(2):
                        pp = psm.tile([P, TS], F32, space="PSUM", tag="mm")
                        for k in range(DC):
                            nc.tensor.matmul(
                                pp[:], wq_sb[k][:, m * P:(m + 1) * P], aw[k][w][:],
                                start=(k == 0), stop=(k == DC - 1))
                        if m == 0:
                            nc.scalar.copy(qk_sb[m][:, w * TS:(w + 1) * TS], pp[:])
                        else:
                            nc.vector.tensor_copy(
                                qk_sb[m][:, w * TS:(w + 1) * TS], pp[:])

                # ---- V^T tiles (token-partition), both heads at once ----
                for kj in range(TC):
                    wj, jj = kj // 4, kj % 4
                    vt = psm.tile([P, TS], F32, space="PSUM", tag="mm")
                    for k in range(DC):
                        nc.tensor.matmul(
                            vt[:, :P],
                            aw[k][wj][:, jj * P:(jj + 1) * P],
                            wq_sb[k][:, 2 * P:3 * P],
                            start=(k == 0), stop=(k == DC - 1))
                    nc.scalar.copy(vx[kj][0][:, :DK], vt[:, :DK])
                    nc.vector.tensor_copy(vx[kj][1][:, :DK], vt[:, DK:2 * DK])

                # ---- attention: scores -> exp -> mask -> (V^T|1) @ P ----
                # After each window, the partial Wo product (contraction over
                # this core's 128 head dims) is computed and staged for a
                # ReduceScatter that sums partials over ranks and scatters
                # token windows -- core r receives Wo-out[:, r's tokens].
                rs_in = dram.tile([4 * D, TS], BF16, tag="rs_in",
                                  name=f"rsi_{l}")
                rs_out = dram.tile([D, TS], BF16, tag="rs_out",
                                   name=f"rso_{l}")
                uT = utp.tile([P, T], BF16, tag="uT", name=f"uT_{l}")
                for w in range(TW):
                    qsl = slice(w * TS, (w + 1) * TS)
                    for h in range(H_PER):
                        hs = slice(h * DK, (h + 1) * DK)
                        nkj = 4 * (w + 1)
                        # kj pairs share one [P, 2*TS] PSUM tile and one Exp
                        pts = []
                        for pi in range(nkj // 2):
                            sp = pss.tile([P, 2 * TS], F32, space="PSUM",
                                          tag="sc")
                            for sub in range(2):
                                kj = 2 * pi + sub
                                nc.tensor.matmul(
                                    sp[:, sub * TS:(sub + 1) * TS],
                                    qk_sb[1][hs, kj * P:(kj + 1) * P],
                                    qk_sb[0][hs, qsl], start=True, stop=True)
                            pt = ptp.tile([P, 2 * TS], BF16, tag="pt")
                            nc.scalar.activation(
                                pt[:], sp[:], mybir.ActivationFunctionType.Exp,
                                bias=0.0, scale=0.125)
                            for sub in range(2):
                                kj = 2 * pi + sub
                                if kj >= 4 * w:
                                    off = kj * P - w * TS
                                    nc.gpsimd.affine_select(
                                        out=pt[:, sub * TS:(sub + 1) * TS],
                                        in_=pt[:, sub * TS:(sub + 1) * TS],
                                        compare_op=mybir.AluOpType.is_ge,
                                        fill=0.0, base=-off,
                                        pattern=[[1, TS]], channel_multiplier=-1)
                            pts.append(pt)
                        pa = psatt.tile([DK + 1, TS], F32, space="PSUM", tag="att")
                        for kj in range(nkj):
                            nc.tensor.matmul(
                                pa[:], vx[kj][h][:],
                                pts[kj // 2][:, (kj % 2) * TS:(kj % 2 + 1) * TS],
                                start=(kj == 0), stop=(kj == nkj - 1))
                        den = strow.tile([1, TS], F32, tag="den")
                        nc.vector.tensor_copy(den[:], pa[DK:DK + 1, :])
                        pb = psm.tile([P, TS], F32, space="PSUM", tag="mm",
                                      name=f"pb{l}_{w}_{h}")
                        nc.tensor.matmul(pb[:DK, :], ones_row[:, :DK], den[:],
                                         start=True, stop=True)
                        rec = recp.tile([DK, TS], F32, tag="rec")
                        nc.vector.reciprocal(rec[:], pb[:DK, :])
                        nc.vector.tensor_tensor(
                            out=uT[hs, qsl], in0=pa[:DK, :], in1=rec[:],
                            op=mybir.AluOpType.mult)
                    # partial Wo for this window, staged to the RS buffer
                    for m in range(DC):
                        pw = psm.tile([P, TS], F32, space="PSUM", tag="mm")
                        nc.tensor.matmul(
                            pw[:], wo_sb[:, m * P:(m + 1) * P], uT[:, qsl],
                            start=True, stop=True)
                        wop = utwp.tile([P, TS], BF16, tag="wop",
                                        name=f"wop{l}_{w}_{m}")
                        nc.vector.tensor_copy(wop[:], pw[:])
                        nc.sync.dma_start(
                            rs_in[w * D + m * P:w * D + (m + 1) * P, :], wop[:])

                nc.gpsimd.collective_compute(
                    "ReduceScatter", mybir.AluOpType.add,
                    replica_groups=GROUPS,
                    ins=[rs_in[:].opt()], outs=[rs_out[:].opt()])

                # ---- residual on my token slice ----
                delta = [dlt.tile([P, TS], BF16, tag=f"dl{m}", name=f"dl{m}_{l}")
                         for m in range(DC)]
                hmid = [hmidp.tile([P, TS], F32, tag=f"hm{m}", name=f"hm{m}_{l}")
                        for m in range(DC)]
                for m in range(DC):
                    nc.sync.dma_start(delta[m][:], rs_out[m * P:(m + 1) * P, :])
                    nc.vector.tensor_add(hmid[m][:], h_cur[m][:], delta[m][:])

                # ---- LN2 + FFN on my token slice ----
                a2 = [a2p.tile([P, TS], BF16, tag=f"a2_{c}", name=f"a2{c}_{l}")
                      for c in range(DC)]
                ln_apply(hmid, g2v[l], b2lv[l], a2, f"ln2_{l}")
                z1g = [z1p.tile([P, TS], BF16, tag=f"z1_{m}", name=f"z1{m}_{l}")
                       for m in range(FC)]
                for m in range(FC):
                    pp = psm.tile([P, TS], F32, space="PSUM", tag="mm")
                    for k in range(DC):
                        nc.tensor.matmul(
                            pp[:], w1_sb[k][:, m * P:(m + 1) * P], a2[k][:],
                            start=(k == 0), stop=(k == DC - 1))
                    nc.scalar.activation(
                        z1g[m][:], pp[:], mybir.ActivationFunctionType.Gelu,
                        bias=fb1v[l][:, m:m + 1])
                h_new = [hpool.tile([P, TS], F32, tag=f"h{c}", name=f"h{c}_{l + 1}")
                         for c in range(DC)]
                for md in range(DC):
                    pp = psm.tile([P, TS], F32, space="PSUM", tag="mm")
                    for k in range(FC):
                        nc.tensor.matmul(
                            pp[:], w2_sb[k][:, md * P:(md + 1) * P], z1g[k][:],
                            start=(k == 0), stop=(k == FC - 1))
                    tt = smallp.tile([P, TS], F32, tag="ffn_out")
                    nc.vector.tensor_scalar(
                        out=tt[:], in0=pp[:], scalar1=fb2v[l][:, md:md + 1],
                        scalar2=None, op0=mybir.AluOpType.add)
                    nc.vector.tensor_add(h_new[md][:], hmid[md][:], tt[:])

                # ---- next-layer LN1 (or final LN) + AllGather ----
                if l + 1 < L:
                    aw = emit_aw_ag(h_new, g1v[l + 1], b1lv[l + 1], f"l{l + 1}in")
                else:
                    aw = emit_aw_ag(h_new, gfv, bfv, "fin")
                h_cur = h_new

            # ---- vocab-shard projection from gathered final-LN activations ----
            # vocab chunks of 1024 (plus an 832 tail) share one [P, 2*TS]
            # PSUM tile; the two 512-col halves are bank-aligned so their
            # accumulation groups don't clobber each other, and the PSUM
            # drain + logits DMA run at [P, 1024] granularity.
            chunks = [(i * 1024, 1024) for i in range(VSH // 1024)]
            if VSH % 1024:
                chunks.append((VSH - VSH % 1024, VSH % 1024))
            for ci, (v0, vw) in enumerate(chunks):
                ow_sb = [owp.tile([P, 1024], BF16, tag="ow", name=f"ow{ci}_{k2}")
                         for k2 in range(DC)]
                for k in range(DC):
                    nc.gpsimd.dma_start(
                        ow_sb[k][:, :vw], outwT[k * P:(k + 1) * P, v0:v0 + vw])
                for tcx in range(TC):
                    r, j = tcx // 4, tcx % 4
                    pp = pss.tile([P, 2 * TS], F32, space="PSUM", tag="sc")
                    for h0, hw in ((0, TS), (TS, vw - TS)):
                        for k in range(DC):
                            nc.tensor.matmul(
                                pp[:, h0:h0 + hw],
                                aw[k][r][:, j * P:(j + 1) * P],
                                ow_sb[k][:, h0:h0 + hw],
                                start=(k == 0), stop=(k == DC - 1))
                    lo = smallp.tile([P, 1024], BF16, tag="lo",
                                     name=f"lo{ci}_{tcx}")
                    if tcx % 2 == 0:
                        nc.scalar.copy(lo[:, :vw], pp[:, :vw])
                    else:
                        nc.vector.tensor_copy(lo[:, :vw], pp[:, :vw])
                    nc.sync.dma_start(
                        logits[tcx * P:(tcx + 1) * P, v0:v0 + vw],
                        lo[:, :vw])
    nc.compile()
    return nc


_NC_CACHE = None


def _get_nc():
    global _NC_CACHE
    if _NC_CACHE is None:
        _NC_CACHE = build_nc_full()
    return _NC_CACHE


def _vec_tile(v, chunks):
    # [chunks*128] -> [128, chunks] with [p, c] = v[c*128+p]
    return np.ascontiguousarray(np.asarray(v, np.float32).reshape(chunks, P).T)


def prepare_in_maps(inputs):
    return _prep(**inputs)


def _prep(x, embed_w, pos_w, ln1_g, ln1_b, Wqkv, Wo, ln2_g, ln2_b,
          W1, b1, W2, b2, lnf_g, lnf_b, out_w):
    x = np.asarray(x)
    embed_w = np.asarray(embed_w, np.float32)
    pos_w = np.asarray(pos_w, np.float32)
    Wqkv = np.asarray(Wqkv, np.float32)
    bf = ml_dtypes.bfloat16
    woT_full = np.ascontiguousarray(np.asarray(Wo).transpose(0, 2, 1)).astype(bf)
    w1T = np.ascontiguousarray(np.asarray(W1).transpose(0, 2, 1)).astype(bf)
    w2T = np.ascontiguousarray(np.asarray(W2).transpose(0, 2, 1)).astype(bf)
    ln_tiles = {
        "ln1g": np.stack([_vec_tile(np.asarray(ln1_g)[l], DC) for l in range(L)]),
        "ln1b": np.stack([_vec_tile(np.asarray(ln1_b)[l], DC) for l in range(L)]),
        "ln2g": np.stack([_vec_tile(np.asarray(ln2_g)[l], DC) for l in range(L)]),
        "ln2b": np.stack([_vec_tile(np.asarray(ln2_b)[l], DC) for l in range(L)]),
        "b1v": np.stack([_vec_tile(np.asarray(b1)[l], FC) for l in range(L)]),
        "b2v": np.stack([_vec_tile(np.asarray(b2)[l], DC) for l in range(L)]),
        "lnfg": _vec_tile(lnf_g, DC),
        "lnfb": _vec_tile(lnf_b, DC),
    }
    in_maps = []
    for c in range(N_CORES):
        b, r = c // 4, c % 4
        h0 = embed_w[x[b]] + pos_w[:T]                       # [T, D]
        h0T = np.ascontiguousarray(
            h0[r * TS:(r + 1) * TS].T).astype(np.float32)    # [D, TS]
        heads = [2 * r, 2 * r + 1]
        rows = np.concatenate([
            np.r_[heads[0] * DK:(heads[0] + 1) * DK,
                  heads[1] * DK:(heads[1] + 1) * DK] + w * D
            for w in range(3)])
        wqkvT = np.ascontiguousarray(
            Wqkv[:, rows, :].transpose(0, 2, 1)).astype(bf)  # [L, 512, 384]
        woT = np.ascontiguousarray(
            woT_full[:, r * P:(r + 1) * P, :])               # [L, 128, 512]
        outwT = np.ascontiguousarray(
            np.asarray(out_w)[r * VSH:(r + 1) * VSH].T).astype(bf)
        m = {"h0T": h0T, "wqkvT": wqkvT, "woT": woT, "w1T": w1T, "w2T": w2T,
             "outwT": outwT}
        m.update(ln_tiles)
        in_maps.append(m)
    return in_maps


def kernel(**inputs):
    nc = _get_nc()
    in_maps = prepare_in_maps(inputs)
    res = run_bass_kernel_spmd(nc, in_maps, list(range(N_CORES)))
    out = np.empty((2, T, 4 * VSH), np.float32)
    for c in range(N_CORES):
        b, r = c // 4, c % 4
        out[b, :, r * VSH:(r + 1) * VSH] = \
            np.asarray(res.results[c]["logits"]).astype(np.float32)
    return out
